# revision 26
# baseline (speedup 1.0000x reference)
"""Trainium2 Bass kernel for nn_GrassmannNN.

Math: the reference's Z2-graded network collapses per-sample to a chain of
32x32 matmuls selected by the sample's 8 bits, so there are only 256 distinct
outputs.  Per body layer l the two transition matrices are
  M0_l = (sum_{j<16}  e0_l[j] G_l[:,j,:]) * blockdiag_mask
  M1_l = (sum_{j>=16} e1_l[j] G_l[:,j,:]) * antidiag_mask (rows 16: negated)
and x <- tanh(x @ M_b) for bit b; the head is x0 @ (head_w * blockdiag).

Device algorithm (per core, fully replicated table + 1/8 of the batch):
  1. body_w is host-packed (pure gather, fp16) into (112, 1024): rows (l, j),
     free order (k, blk, i) so a 32x32-block stream transpose lands rows right.
  2. Scale rows by e0/e1 columns (2 vector ops), then TWO fp16 matmuls with a
     layer-indicator lhsT produce all 14 M-matrices into one PSUM tile (rows
     0:7 = M0 layers, 7:14 = M1 layers).
  3. nc.vector.transpose (DVE 32x32 stream transpose) moves (blk,i) to
     partitions in one op; 4 strided casting copies scatter the blocks into
     Mpair (32, 7, 2, 32) fp16 = per-layer [M0 | M1] in lhsT layout.
  4. Doubling table build: state X_l (32, 2^l) fp16 over all bit-prefixes; per
     site two matmuls (branch 0/1 into adjacent psum column ranges) + one tanh.
     Site 7 flips orientation (lhsT = state) to land the table pattern-major
     as Th (128, 2, 4, 16) fp16 with the output zero-structure baked in.
  5. Gather: idx = pow2 @ bits (exact in fp16), one-hot via is_equal, then
     out^T (64, 1024) = Th0^T oh0 + Th1^T oh1 in 2x512 psum chunks.
  6. Output is stored transposed fp16 (64, BC); the host unshards/casts.
Filler matmuls keep the PE HAM-warm across the chain so the big gather
matmuls run at 2.4 GHz.
"""

import numpy as np
from contextlib import ExitStack

import concourse.bass as bass
import concourse.bacc as bacc
import concourse.tile as tile
import concourse.mybir as mybir
from concourse.bass_utils import run_bass_kernel_spmd

F32 = mybir.dt.float32
F32R = mybir.dt.float32r
F16 = mybir.dt.float16
I32 = mybir.dt.int32
AF = mybir.ActivationFunctionType
OP = mybir.AluOpType

NCORES = 8
DEBUG = False
CAST_SWAP = True          # chunk-1 output cast on DVE (forces is_equal order)
B = 8192
BC = B // NCORES          # 1024 samples per core

# bodypk (fp32): cols 0:512 M0 blocks, 512:1024 M1 blocks, free order (k, blk, i)
C_M0 = 0
C_M1 = 512
C_TOT = 1024
# aux (fp32) columns
A_E0 = 0                  # cols 0:7   E0 = e0 scattered onto the layer blocks
A_E1 = 7                  # cols 7:14  E1 likewise
A_MBD = 14                # cols 14:46 maskbd rows 0:32
A_EMB0 = 46               # site-0 embedding col (rows 0:32)
A_HW = 47                 # cols 47:79 head_w rows 0:32
A_TOT = 79


def _host_consts():
    ind = np.zeros((112, 7), np.float32)
    for l in range(7):
        ind[16 * l:16 * l + 16, l] = 1.0
    maskbd = np.zeros((112, 32), np.float32)
    maskbd[:16, :16] = 1.0
    maskbd[16:32, 16:] = 1.0
    pw2 = np.zeros((112, 1), np.float32)
    pw2[:8, 0] = (1 << np.arange(8)).astype(np.float32)
    return ind, maskbd, pw2


def _emit(ctx: ExitStack, tc, t):
    nc = tc.nc
    body_d, aux_d, datat_d, out_d = t["bodypk"], t["aux"], t["dataT"], t["out"]

    cpool = ctx.enter_context(tc.tile_pool(name="consts", bufs=1))
    work = ctx.enter_context(tc.tile_pool(name="work", bufs=1))
    psum = ctx.enter_context(tc.tile_pool(name="psum", bufs=1, space="PSUM"))

    # ---- input DMAs across both HWDGE queues; small/early regions first ----
    bodyT = cpool.tile([112, C_TOT], F32)
    auxT = cpool.tile([112, A_TOT], F32)
    nc.sync.dma_start(bodyT[:, C_M0:C_M1], body_d.ap()[:, C_M0:C_M1])
    nc.scalar.dma_start(auxT[:], aux_d.ap())
    nc.scalar.dma_start(bodyT[:, C_M1:C_TOT], body_d.ap()[:, C_M1:C_TOT])
    dTb = cpool.tile([8, BC + 128], F16)
    nc.sync.dma_start(dTb[:], datat_d.ap())

    # ---- small constants / zero-fills while the DMAs land ----
    warm = cpool.tile([128, 256], F16)
    nc.gpsimd.memset(warm[:], 0.0)
    iotaI = cpool.tile([128, 1], I32)
    nc.gpsimd.iota(iotaI[:], [[0, 1]], base=0, channel_multiplier=1)
    iotaF = cpool.tile([128, 1], F32)
    nc.vector.tensor_copy(iotaF[:], iotaI[:])
    Mpair = work.tile([32, 7, 2, 32], F32)
    Th = work.tile([128, 2, 4, 16], F16)
    nc.vector.memset(Th[:, :, 1:3, :], 0.0)
    psW = psum.tile([64, 16, 32], F32, tag="w")
    nc.vector.memset(psW[:], 0.0)

    # ---- PE clock warmup: dummy matmuls while input DMAs land ----
    pwarm = psum.tile([128, 256], F32, tag="warm")
    for _ in range(14):
        nc.tensor.matmul(pwarm[:], warm[:, 0:128], warm[:, 0:256],
                         start=True, stop=True)

    # dummy activation: pull the ACT table load off the critical path
    actwarm = cpool.tile([1, 8], F32)
    nc.vector.memset(actwarm[:], 0.0)
    nc.scalar.activation(actwarm[:], actwarm[:], AF.Tanh)

    # ---- head constants (E0/E1/pow2row come host-packed) ----
    Mh = work.tile([32, 32], F32)
    nc.vector.tensor_mul(Mh[:], auxT[0:32, A_HW:A_HW + 32],
                         auxT[0:32, A_MBD:A_EMB0])
    X0 = work.tile([32, 2], F32)
    nc.scalar.activation(X0[:], auxT[0:32, A_MBD:A_MBD + 17:16], AF.Copy,
                         scale=auxT[0:32, A_EMB0:A_EMB0 + 1])

    # ---- head: X1 = tanh(Mh^T X0) (runs while the big DMAs land) ----
    ps = psum.tile([32, 256], F32, tag="chain")
    nc.tensor.matmul(ps[:, 0:2], Mh[:], X0[:], start=True, stop=True)
    X = work.tile([32, 2], F32, tag="X1")
    nc.scalar.activation(X[:], ps[:, 0:2], AF.Tanh)
    xs_dbg = [X]

    # ---- W build: psW rows 0:7 = M0 layers (e0), rows 7:14 = M1 (e1) ----
    nc.tensor.matmul(psW[0:7, :, :].rearrange("p a b -> p (a b)"),
                     auxT[:, A_E0:A_E1], bodyT[:, C_M0:C_M1],
                     start=True, stop=True)
    nc.tensor.matmul(psW[32:39, :, :].rearrange("p a b -> p (a b)"),
                     auxT[:, A_E1:A_MBD], bodyT[:, C_M1:C_TOT],
                     start=True, stop=True)

    # ---- 32x32-block stream transpose: Wt[(blk,i), k, lam] = psW[lam, k, blk*16+i]
    Wt = work.tile([64, 16, 32], F32)
    nc.vector.transpose(Wt[:].rearrange("p a b -> p (a b)"),
                        psW[:].rearrange("p a b -> p (a b)"))

    # ---- scatter blocks into per-layer lhsT form Mpair (32, l, sel, k) ----
    # Full-32-partition masked copies: the maskbd column zeroes the rows that
    # belong to the other 16-block, so each Mpair position is written once.
    for sel in (0, 1):
        for ch in (0, 1):
            blkv = ch ^ sel
            msk = auxT[0:32, A_MBD + 16 * blkv:A_MBD + 16 * blkv + 1]
            srcv = Wt[32 * sel:32 * sel + 32, :, 0:7]
            dst = Mpair[0:32, :, sel, 16 * ch:16 * ch + 16]
            dst = dst.rearrange("p l k -> p k l")
            if ch == 1:
                nc.scalar.activation(dst, srcv, AF.Copy, scale=msk)
            else:
                nc.vector.tensor_scalar(dst, srcv, msk, None, OP.mult)
    # layer 6 in fp16 for the cheap single-pass site-7 matmul
    Mp6 = work.tile([32, 2, 32], F16)
    nc.gpsimd.tensor_copy(Mp6[:], Mpair[:, 6, :, :])

    # ---- one-hot path: idx = sum_j 2^j b_j, broadcast via matmul ----
    oh0 = cpool.tile([128, BC], F16)
    oh1 = cpool.tile([128, BC], F16)
    pidxt = psum.tile([128, BC], F32, tag="idx")
    for c in range(2):
        nc.tensor.matmul(pidxt[:, c * 512:(c + 1) * 512], dTb[:, BC:BC + 128],
                         dTb[:, c * 512:(c + 1) * 512],
                         start=True, stop=True)
    pidx = [pidxt[:, 0:512], pidxt[:, 512:1024]]

    # ---- doubling chain, sites 1..6 (+ PE keep-warm fillers) ----
    for ml in range(6):
        N = 2 << ml
        ps = psum.tile([32, 256], F32, tag="chain")
        for b in range(2):
            nc.tensor.matmul(ps[:, b * N:(b + 1) * N],
                             Mpair[:, ml, b, :], X[:],
                             start=True, stop=True)
        X = work.tile([32, 2 * N], F16 if ml == 5 else F32, tag=f"X{ml + 2}")
        nc.scalar.activation(X[:], ps[:, 0:2 * N], AF.Tanh)
        xs_dbg.append(X)

    # one-hot compares on DVE; hold them past the Mpair copies so the
    # scheduler cannot slot them onto DVE ahead of the critical-path scatter
    with tc.tile_wait_until(0.012):
        for c in range(2):
            sl = slice(c * 512, (c + 1) * 512)
            nc.vector.tensor_scalar(oh0[:, sl], pidx[c], iotaF[:, 0:1],
                                    None, OP.is_equal)
            nc.vector.tensor_scalar(oh1[:, sl], pidx[c], 128.0,
                                    iotaF[:, 0:1], OP.subtract, OP.is_equal)

    # ---- site 7: land pattern-major, bake output zero structure ----
    pf = psum.tile([128, 2, 2, 16], F32, tag="pf")
    nc.tensor.matmul(pf[:].rearrange("p s a b -> p (s a b)"),
                     X[:], Mp6[:], start=True, stop=True)
    nc.scalar.activation(Th[:, :, 0:4:3, :], pf[:], AF.Tanh)

    # ---- gather: out^T = Th0^T oh0 + Th1^T oh1, in 512-sample chunks ----
    OUT = work.tile([64, BC], F16)
    pgt = psum.tile([64, BC], F32, tag="g")
    for c in range(2):
        sl = slice(c * 512, (c + 1) * 512)
        pg = pgt[:, sl]
        nc.tensor.matmul(pg, Th[:, 0, :, :].rearrange("p a b -> p (a b)"),
                         oh0[:, sl], start=True, stop=False)
        nc.tensor.matmul(pg, Th[:, 1, :, :].rearrange("p a b -> p (a b)"),
                         oh1[:, sl], start=False, stop=True)
        if c == (0 if CAST_SWAP else 1):
            nc.scalar.copy(OUT[:, sl], pg)
        else:
            nc.vector.tensor_copy(OUT[:, sl], pg)
        if c == 0:
            nc.sync.dma_start(out_d.ap()[:, sl], OUT[:, sl])
        else:
            nc.scalar.dma_start(out_d.ap()[:, sl], OUT[:, sl])
    if DEBUG:
        nc.sync.dma_start(t["dbg_w"].ap(), Wt[:].rearrange("p a b -> p (a b)"))
        nc.sync.dma_start(t["dbg_mpair"].ap(),
                          Mpair[:].rearrange("p a b c -> p (a b c)"))
        nc.sync.dma_start(t["dbg_th"].ap(),
                          Th[:].rearrange("p s a b -> p (s a b)"))
        nc.sync.dma_start(t["dbg_oh"].ap()[:, 0:64], oh0[:, 0:64])
        nc.sync.dma_start(t["dbg_oh"].ap()[:, 64:128], oh1[:, 0:64])
        pass
        off = 0
        for xd in xs_dbg[:6]:
            n = xd.shape[1]
            nc.scalar.dma_start(t["dbg_xs"].ap()[:, off:off + n], xd[:])
            off += n


def build_program():
    nc = bacc.Bacc("TRN2", target_bir_lowering=False, debug=False,
                   enable_asserts=False, num_devices=NCORES,
                   enable_partition_id=False)
    t = {}
    t["bodypk"] = nc.dram_tensor("bodypk", [112, C_TOT], F32, kind="ExternalInput")
    t["aux"] = nc.dram_tensor("aux", [112, A_TOT], F32, kind="ExternalInput")
    t["dataT"] = nc.dram_tensor("dataT", [8, BC + 128], F16, kind="ExternalInput")
    t["out"] = nc.dram_tensor("out", [64, BC], F16, kind="ExternalOutput")
    if DEBUG:
        t["dbg_w"] = nc.dram_tensor("dbg_w", [64, 512], F32, kind="ExternalOutput")
        t["dbg_mpair"] = nc.dram_tensor("dbg_mpair", [32, 448], F32, kind="ExternalOutput")
        t["dbg_th"] = nc.dram_tensor("dbg_th", [128, 128], F16, kind="ExternalOutput")
        t["dbg_oh"] = nc.dram_tensor("dbg_oh", [128, 128], F16, kind="ExternalOutput")

        t["dbg_xs"] = nc.dram_tensor("dbg_xs", [32, 126], F32, kind="ExternalOutput")
    with tile.TileContext(nc) as tc:
        with ExitStack() as ctx:
            _emit(ctx, tc, t)
    nc.compile()
    return nc


def make_in_maps(data, embedding, head_w, body_w):
    data = np.asarray(data)
    if data.dtype == np.int64:
        d32 = data.view(np.int32).reshape(B, 16)[:, ::2]
    else:
        d32 = data.astype(np.int32, copy=False)
    embedding = np.asarray(embedding, np.float32)
    head_w = np.asarray(head_w, np.float32)
    body_w = np.asarray(body_w, np.float32)

    ind, maskbd, pw2 = _host_consts()
    bodypk = np.zeros((112, C_TOT), np.float32)
    # free order (k, blk, i): col = k*32 + blk*16 + i
    l_, j_, k_, blk_, i_ = np.ix_(np.arange(7), np.arange(16), np.arange(16),
                                  np.arange(2), np.arange(16))
    bodypk[:, C_M0:C_M1] = body_w[l_, i_ + 16 * blk_, j_, k_ + 16 * blk_
                                  ].reshape(112, 512)
    # blk=1 half (the D block) carries the graded sign: pack negated
    m1 = body_w[l_, i_ + 16 * blk_, 16 + j_, k_ + 16 * (1 - blk_)]
    m1[:, :, :, 1, :] *= -1.0
    bodypk[:, C_M1:C_TOT] = m1.reshape(112, 512)

    aux = np.zeros((112, A_TOT), np.float32)
    e0v = embedding[1:, 0, :].reshape(112)
    e1v = embedding[1:, 1, :].reshape(112)
    rows = np.arange(112)
    aux[rows, A_E0 + rows // 16] = e0v
    aux[rows, A_E1 + rows // 16] = e1v
    aux[:, A_MBD:A_EMB0] = maskbd
    aux[0:32, A_EMB0] = embedding[0].reshape(32)
    aux[0:32, A_HW:A_HW + 32] = head_w

    base = {"bodypk": bodypk, "aux": aux}
    in_maps = []
    pw2row = np.broadcast_to((1 << np.arange(8, dtype=np.int64))[:, None],
                              (8, 128)).astype(np.float16)
    for c in range(NCORES):
        dslice = np.concatenate(
            [d32[c * BC:(c + 1) * BC].T.astype(np.float16), pw2row], axis=1)
        in_maps.append({**base, "dataT": np.ascontiguousarray(dslice)})
    return in_maps


def postprocess(results):
    # per-core out is (64, BC) transposed fp16; unshard to (B, 2, 32) fp32
    full = np.concatenate(
        [np.ascontiguousarray(results[c]["out"].T.astype(np.float32))
         for c in range(NCORES)],
        axis=0)
    return full.reshape(B, 2, 32)


_CACHE = {}


def kernel(data, embedding, head_w, body_w, **kw):
    nc = _CACHE.get("nc")
    if nc is None:
        nc = build_program()
        _CACHE["nc"] = nc
    in_maps = make_in_maps(data, embedding, head_w, body_w)
    res = run_bass_kernel_spmd(nc, in_maps, core_ids=list(range(NCORES)))
    return postprocess(res.results)
